# revision 30
# baseline (speedup 1.0000x reference)
"""GAT-style bipartite graph attention layer (nn_BiGraphContrastLayer) on 8 trn2 cores.

Strategy (dst-sharded SPMD, one shared program, per-core node renumbering):
  - Each core works with a LOCAL node table of NLOC=11280 rows:
    rows [0, 1280): its own 1250 dst nodes, permuted into 10 balanced bins
    of 128 (padded with zero rows); rows [1280, 11280): all 10000 src nodes.
    The host permutes each core's xT input accordingly, so the shared
    program uses identical (static) access patterns on every core.
  - Phase 1: zel_tab[n] = [z(512) | el(8) | er(8)] bf16, z = x @ W etc., for
    the 11520-row padded local table.  PSUM->SBUF casts alternate
    Vector/Scalar engines.
  - Phase 2 per dst bin (10 bins): incoming edges, sorted by src and split
    into a LO half (first K_LO edges; src rows < w_tiles row-tiles) and a
    HI half, are fetched from zel_tab with SWDGE dma_gather in prepare_only
    mode + trigger_dma on 2 queues.  Descriptor generation overlaps phase 1;
    LO triggers depend only on the first w_tiles zel row-tiles, so LO
    transfers and compute start while phase 1 is still finishing.
    Self-loops use static DMAs (own-dst rows sit at fixed offsets) with an
    identity selection matrix.  v = exp(leaky_relu(el_src + er_dst));
    one-hot fp8 selection matmuls segment-sum messages and weights in PSUM.
    out = po/s + bias.
  No inter-core communication; the host unpermutes the 8 dst slices.
"""
import os

import numpy as np
import ml_dtypes

import concourse.bacc as bacc
import concourse.bass as bass
import concourse.mybir as mybir
import concourse.tile as tile
from concourse.instruction_name_ordered_set import InstructionNameOrderedSet

BF = ml_dtypes.bfloat16
F32 = np.float32
F8 = ml_dtypes.float8_e4m3fn

NS, ND, E, DIN, H, DH = 10000, 10000, 320000, 512, 8, 64
NEG = 0.2
NCORES = 8
DPC = ND // NCORES          # 1250 dst nodes per core
NTILES = 10                 # dst bins per core (128 slots each)
NSLOT = NTILES * 128        # 1280 dst slots
NLOC = NSLOT + NS           # local node rows: own dsts | all srcs
NPAD2 = 11520               # padded to 90 tiles of 128 (9 panels of 1280)
PAD_ROW = NLOC              # all-zero row used by pad gather indices
ROW = 640                   # zel row stride in elems (1280B, 256B-aligned)
PANEL = 1280                # phase-1 node panel (10 subtiles of 128)
K_LO = 2048                 # lo-half edges per bin (16 chunks, no pads)
NCH_LO = K_LO // 128
POOLCH = 8                  # hi-half chunks whose v*z multiply runs on Pool


# ----------------------------------------------------------------- host prep
def _wrap_idx(idx):
    """dma_gather index layout: idx i -> [i % 16, i // 16], replicated 8x."""
    k = len(idx)
    w = np.zeros((16, k // 16), np.int16)
    w[np.arange(k) % 16, np.arange(k) // 16] = idx
    return np.tile(w, (8, 1))


def _host_prep(x_src, x_dst, edge_src, edge_dst, W, attn_l, attn_r, bias):
    Al = np.zeros((DIN, H), F32)
    Ar = np.zeros((DIN, H), F32)
    for h in range(H):
        Al[h * DH:(h + 1) * DH, h] = attn_l[h]
        Ar[h * DH:(h + 1) * DH, h] = attn_r[h]
    Wext = np.concatenate([W, W @ Al, W @ Ar], 1).astype(BF)  # [512, 528]
    bias_rep = np.tile(bias[None, :].astype(F32), (128, 1))   # [128, 512]
    ident = np.eye(128, dtype=F8)                             # [128, 128]

    edge_src = edge_src.astype(np.int64)
    edge_dst = edge_dst.astype(np.int64)

    # first pass: per-core balanced bin assignment -> global k_tile
    per_core_raw = []
    kmax = 0
    kmin = 1 << 30
    for c in range(NCORES):
        d0 = c * DPC
        m = (edge_dst >= d0) & (edge_dst < d0 + DPC)
        es = edge_src[m]
        ed = edge_dst[m] - d0
        deg = np.bincount(ed, minlength=DPC)
        # LPT greedy: heaviest nodes first onto least-loaded feasible bin
        order = np.argsort(-deg, kind="stable")
        bin_nodes = [[] for _ in range(NTILES)]
        bin_load = np.zeros(NTILES, np.int64)
        for d in order:
            feas = [b for b in range(NTILES) if len(bin_nodes[b]) < 128]
            b = min(feas, key=lambda b: bin_load[b])
            bin_nodes[b].append(d)
            bin_load[b] += deg[d]
        kmax = max(kmax, int(bin_load.max()))
        kmin = min(kmin, int(bin_load.min()))
        per_core_raw.append((es, ed, bin_nodes))
    assert kmin >= K_LO, f"bin with {kmin} < {K_LO} edges"
    k_tile = ((kmax + 127) // 128) * 128
    k_hi = k_tile - K_LO
    nch_hi = k_hi // 128
    nch = NCH_LO + nch_hi

    w_split_row = 0  # max zel row touched by any lo-half (exclusive)
    per_core = []
    for c in range(NCORES):
        es, ed, bin_nodes = per_core_raw[c]
        perm = np.full(NSLOT, -1, np.int64)      # slot -> local dst id
        slot_of = np.full(DPC, -1, np.int64)     # local dst id -> slot
        for b in range(NTILES):
            for j, d in enumerate(bin_nodes[b]):
                s = b * 128 + j
                perm[s] = d
                slot_of[d] = s
        eslot = slot_of[ed]
        ebin = eslot // 128

        klo16, khi16 = K_LO // 16, k_hi // 16
        zidx_lo = np.zeros((128, NTILES * klo16), np.int16)
        zidx_hi = np.zeros((128, NTILES * khi16), np.int16)
        selT = np.zeros((128, NTILES * nch * 128), F8)
        selD = np.zeros((128, NTILES * nch * 128), F8)
        for t in range(NTILES):
            sel_e = ebin == t
            srcs = es[sel_e]
            slots = eslot[sel_e] - t * 128
            o = np.argsort(srcs, kind="stable")   # src-sorted within bin
            srcs, slots = srcs[o], slots[o]
            k = len(srcs)
            rows = np.full(k_tile, PAD_ROW, np.int64)
            rows[:k] = NSLOT + srcs
            w_split_row = max(w_split_row, int(rows[K_LO - 1]) + 1)
            zidx_lo[:, t * klo16:(t + 1) * klo16] = _wrap_idx(rows[:K_LO])
            zidx_hi[:, t * khi16:(t + 1) * khi16] = _wrap_idx(rows[K_LO:])
            sl = np.full(k_tile, -1, np.int64)
            sl[:k] = slots
            for ch in range(nch):
                d = sl[ch * 128:(ch + 1) * 128]
                sm = np.zeros((128, 128), F32)
                valid = d >= 0
                sm[np.arange(128)[valid], d[valid]] = 1.0
                j = (t * nch + ch) * 128
                selT[:, j:j + 128] = sm.astype(F8)
                selD[:, j:j + 128] = sm.T.astype(F8)

        # permuted xT: cols [0,1280) own dsts (pads zero), [1280,11280) srcs
        xT = np.zeros((DIN, NPAD2), BF)
        dst_cols = x_dst[c * DPC + perm]
        dst_cols[perm < 0] = 0.0
        xT[:, :NSLOT] = dst_cols.T
        xT[:, NSLOT:NLOC] = x_src.T
        per_core.append(dict(xT=xT, selT=selT, selD=selD, zidx_lo=zidx_lo,
                             zidx_hi=zidx_hi, perm=perm))

    w_panels = (w_split_row + 639) // 640  # zel half-panels lo-halves need
    shared = dict(Wext=Wext, bias_rep=bias_rep, ident=ident)
    return shared, per_core, k_hi, w_panels


# ------------------------------------------------------------- bass program
def _build_nc(k_hi, w_panels):
    nch_hi = k_hi // 128
    nch = NCH_LO + nch_hi
    nc = bacc.Bacc("TRN2", target_bir_lowering=False, debug=False,
                   num_swdge_queues=4)
    dt = mybir.dt

    xT_d = nc.dram_tensor("xT", [DIN, NPAD2], dt.bfloat16, kind="ExternalInput")
    W_d = nc.dram_tensor("Wext", [DIN, 528], dt.bfloat16, kind="ExternalInput")
    bias_d = nc.dram_tensor("bias_rep", [128, 512], dt.float32,
                            kind="ExternalInput")
    ident_d = nc.dram_tensor("ident", [128, 128], dt.float8e4,
                             kind="ExternalInput")
    selT_d = nc.dram_tensor("selT", [128, NTILES * nch * 128], dt.float8e4,
                            kind="ExternalInput")
    selD_d = nc.dram_tensor("selD", [128, NTILES * nch * 128], dt.float8e4,
                            kind="ExternalInput")
    zlo_d = nc.dram_tensor("zidx_lo", [128, NTILES * K_LO // 16], dt.int16,
                           kind="ExternalInput")
    zhi_d = nc.dram_tensor("zidx_hi", [128, NTILES * k_hi // 16], dt.int16,
                           kind="ExternalInput")
    out_d = nc.dram_tensor("out", [NSLOT, 512], dt.float32,
                           kind="ExternalOutput")
    zel_d = nc.dram_tensor("zel_tab", [NPAD2, ROW], dt.bfloat16)

    glo = [nc.alloc_semaphore(f"glo{t}") for t in range(NTILES)]
    ghi = [nc.alloc_semaphore(f"ghi{t}") for t in range(NTILES)]

    with tile.TileContext(nc) as tc:
        with tc.tile_pool(name="const", bufs=1) as cpool:
            wsb = cpool.tile([128, 4 * 528], dt.bfloat16)
            for k in range(4):
                nc.sync.dma_start(wsb[:, k * 528:(k + 1) * 528],
                                  W_d[k * 128:(k + 1) * 128, :])
            bias_sb = cpool.tile([128, 512], dt.float32)
            nc.sync.dma_start(bias_sb[:], bias_d[:])
            ident_sb = cpool.tile([128, 128], dt.float8e4)
            nc.sync.dma_start(ident_sb[:], ident_d[:])
            zlo_sb = cpool.tile([128, NTILES * K_LO // 16], dt.int16)
            nc.sync.dma_start(zlo_sb[:], zlo_d[:])
            zhi_sb = cpool.tile([128, NTILES * k_hi // 16], dt.int16)
            nc.sync.dma_start(zhi_sb[:], zhi_d[:])
            era = cpool.tile([128, NTILES, 16], dt.bfloat16)

            # phase-2 SBUF pools opened before phase-1 pools: disjoint
            # regions, so early gather preps never alias phase-1 tiles
            p2ctx = (
                tc.tile_pool(name="zglo", bufs=3),
                tc.tile_pool(name="zghi", bufs=3),
                tc.tile_pool(name="zsf", bufs=2),
                tc.tile_pool(name="sel", bufs=2),
                tc.tile_pool(name="sc", bufs=3),
                tc.tile_pool(name="eo", bufs=2),
            )
            pools = [p.__enter__() for p in p2ctx]
            zlopool, zhipool, zsfpool, selpool, scpool, eopool = pools

            zel_writers = []

            def _queue(t, lo):
                # LO on queues 0/2, HI on 1/3, alternating by bin parity so
                # per-queue outstanding descriptors stay low (no ring stalls)
                return (0 if lo else 1) + 2 * (t % 2)

            def emit_prep(t, lo):
                if lo:
                    zg = zlopool.tile([128, NCH_LO, ROW], dt.bfloat16)
                    idx = zlo_sb[:, t * K_LO // 16:(t + 1) * K_LO // 16]
                    kk, sem = K_LO, glo[t]
                else:
                    zg = zhipool.tile([128, nch_hi, ROW], dt.bfloat16)
                    idx = zhi_sb[:, t * k_hi // 16:(t + 1) * k_hi // 16]
                    kk, sem = k_hi, ghi[t]
                prep = nc.gpsimd.dma_gather(
                    zg[:], zel_d[:], idx, num_idxs=kk, num_idxs_reg=kk,
                    elem_size=ROW, single_packet=False,
                    prepare_only=True, sem=sem, queue_num=_queue(t, lo)).ins
                # strip zel RAW deps (descgen reads only idxs); they are
                # re-attached to the matching trigger
                keep = [d for d in prep.sync_dependency_names()
                        if d not in zel_writers]
                prep.set_sync_dependencies(InstructionNameOrderedSet(keep))
                return zg

            def emit_trigger(t, lo):
                trig = nc.gpsimd.trigger_dma(
                    count=1, queue_num=_queue(t, lo)).ins
                deps = zel_writers[:w_panels] if lo else zel_writers
                trig.set_sync_dependencies(InstructionNameOrderedSet(
                    list(trig.sync_dependency_names()) + list(deps)))

            zg_lo, zg_hi = {}, {}
            for t in range(3):
                zg_lo[t] = emit_prep(t, True)

            # ---- phase 1: zel_tab = [x@W | x@Wl | x@Wr] for local nodes
            with (
                tc.tile_pool(name="xp", bufs=2) as xpool,
                tc.tile_pool(name="zel", bufs=2) as zpool,
                tc.tile_pool(name="p1", bufs=3, space="PSUM") as p1pool,
                tc.tile_pool(name="p1b", bufs=3, space="PSUM") as p1bpool,
            ):
                xT_v = xT_d[:].rearrange("(k p) n -> p k n", p=128)
                for p in range(NPAD2 // PANEL):
                    xp = xpool.tile([128, 4, PANEL], dt.bfloat16)
                    nc.sync.dma_start(
                        xp[:], xT_v[:, :, p * PANEL:(p + 1) * PANEL])
                    for h in range(2):       # two 640-row half-panel writes
                        zel_sb = zpool.tile([128, 5, 528], dt.bfloat16)
                        for j in range(5):
                            m = h * 5 + j
                            gm = p * (PANEL // 128) + m
                            zps = p1pool.tile([128, 512], dt.float32,
                                              space="PSUM")
                            lps = p1bpool.tile([128, 16], dt.float32,
                                               space="PSUM")
                            for k in range(4):
                                lhsT = xp[:, k, m * 128:(m + 1) * 128]
                                nc.tensor.matmul(
                                    zps[:], lhsT,
                                    wsb[:, k * 528:k * 528 + 512],
                                    start=(k == 0), stop=(k == 3))
                                nc.tensor.matmul(
                                    lps[:], lhsT,
                                    wsb[:, k * 528 + 512:(k + 1) * 528],
                                    start=(k == 0), stop=(k == 3))
                            if gm % 2 == 0:
                                nc.vector.tensor_copy(
                                    zel_sb[:, j, 0:512], zps[:])
                                vcast = nc.vector.tensor_copy(
                                    zel_sb[:, j, 512:528], lps[:])
                                last_vec_name = vcast.ins.name
                            else:
                                nc.scalar.copy(zel_sb[:, j, 0:512], zps[:])
                                nc.scalar.copy(zel_sb[:, j, 512:528], lps[:])
                        r0 = p * PANEL + h * 640
                        w = nc.sync.dma_start(
                            zel_d[r0:r0 + 640, 0:528]
                            .rearrange("(g p) j -> p g j", p=128),
                            zel_sb[:])
                        zel_writers.append(w.ins.name)

            # el/er of own dst slots: static strided read of rows [0, 1280)
            nc.sync.dma_start(
                era[:],
                zel_d[0:NSLOT, 512:528].rearrange("(t p) j -> p t j", p=128))

            # early lo triggers (fire once the first w_tiles row-tiles land)
            # and hi preps; per-queue FIFO order matches tile order
            for t in range(3):
                emit_trigger(t, True)
            for t in range(3):
                zg_hi[t] = emit_prep(t, False)

            # ---- phase 2: per dst bin gather + attention + aggregation
            with (
                tc.tile_pool(name="p2", bufs=2, space="PSUM") as p2pool,
                tc.tile_pool(name="p2b", bufs=2, space="PSUM") as p2bpool,
                tc.tile_pool(name="p2c", bufs=2, space="PSUM") as p2cpool,
            ):
                for t in range(NTILES):
                    # hi trigger first: prep(t+3) carries a WAR wait on bin-t
                    # PE reads, which transitively need ghi[t] — triggering
                    # before the preps keeps the Pool stream cycle-free
                    emit_trigger(t, False)
                    zgl, zgh = zg_lo[t], zg_hi[t]

                    zsf = zsfpool.tile([128, 512], dt.bfloat16)
                    nc.sync.dma_start(zsf[:],
                                      zel_d[t * 128:(t + 1) * 128, 0:512])
                    sel = selpool.tile([128, nch * 128], dt.float8e4)
                    nc.sync.dma_start(
                        sel[:], selT_d[:, t * nch * 128:(t + 1) * nch * 128])
                    seld = selpool.tile([128, nch * 128], dt.float8e4,
                                        tag="seld")
                    nc.sync.dma_start(
                        seld[:], selD_d[:, t * nch * 128:(t + 1) * nch * 128])

                    # er_dst broadcast to edges via SelD matmuls
                    pe_er = p2cpool.tile([128, nch, 8], dt.float32,
                                         space="PSUM")
                    for ch in range(nch):
                        nc.tensor.matmul(pe_er[:, ch, :],
                                         seld[:, ch * 128:(ch + 1) * 128],
                                         era[:, t, 8:16],
                                         start=True, stop=True,
                                         skip_group_check=True)

                    # el_src + er_dst; the gather-landed waits ride the first
                    # consumers, anchored after prior Vector work so they
                    # cannot park at the head of the Vector queue
                    lt = scpool.tile([128, nch, 8], dt.float32, tag="lt")
                    lt_lo = nc.vector.tensor_tensor(
                        lt[:, 0:NCH_LO, :], zgl[:, :, 512:520],
                        pe_er[:, 0:NCH_LO, :], op=mybir.AluOpType.add)
                    lt_lo._wait_ge(glo[t], 16)
                    lt_lo.ins.set_sync_dependencies(InstructionNameOrderedSet(
                        list(lt_lo.ins.sync_dependency_names())
                        + [last_vec_name]))
                    lt_hi = nc.vector.tensor_tensor(
                        lt[:, NCH_LO:, :], zgh[:, :, 512:520],
                        pe_er[:, NCH_LO:, :], op=mybir.AluOpType.add)
                    lt_hi._wait_ge(ghi[t], 16)
                    lt_hi.ins.set_sync_dependencies(InstructionNameOrderedSet(
                        list(lt_hi.ins.sync_dependency_names())
                        + [lt_lo.ins.name]))
                    nc.vector.scalar_tensor_tensor(
                        lt[:], lt[:], NEG, lt[:],
                        op0=mybir.AluOpType.mult, op1=mybir.AluOpType.max)
                    vb = scpool.tile([128, nch, 8], dt.bfloat16, tag="vb")
                    nc.scalar.activation(vb[:], lt[:],
                                         mybir.ActivationFunctionType.Exp)

                    # self loop: lt = el + er of own slot
                    lts = scpool.tile([128, 8], dt.float32, tag="lts")
                    nc.vector.tensor_tensor(
                        lts[:], era[:, t, 0:8], era[:, t, 8:16],
                        op=mybir.AluOpType.add)
                    nc.vector.scalar_tensor_tensor(
                        lts[:], lts[:], NEG, lts[:],
                        op0=mybir.AluOpType.mult, op1=mybir.AluOpType.max)
                    vbs = scpool.tile([128, 8], dt.bfloat16, tag="vbs")
                    nc.scalar.activation(vbs[:], lts[:],
                                         mybir.ActivationFunctionType.Exp)

                    # msg = v * z.  v is duplicated into adjacent PAIRS
                    # (vbp[.., h, 0:2] = v[h]) so the multiply's b-operand
                    # has a stride-1, 4B-aligned innermost dim — the stride-0
                    # broadcast moves off the innermost axis, letting the DVE
                    # pick its 2x packed mode instead of the 1x fallback.
                    # pair-builds run on the (idle) Scalar engine so the DVE
                    # only sees the fast 2x-mode multiplies
                    vbp = scpool.tile([128, nch, 16], dt.bfloat16, tag="vbp")
                    nc.scalar.copy(
                        vbp[:].rearrange("p c (h two) -> p c h two", two=2),
                        vb[:].rearrange("p c (h one) -> p c h one", one=1)
                        .to_broadcast([128, nch, 8, 2]))
                    vbsp = scpool.tile([128, 16], dt.bfloat16, tag="vbsp")
                    nc.scalar.copy(
                        vbsp[:].rearrange("p (h two) -> p h two", two=2),
                        vbs[:].rearrange("p (h one) -> p h one", one=1)
                        .to_broadcast([128, 8, 2]))

                    z4l = zgl[:, :, 0:512].rearrange(
                        "p c (h g two) -> p c h g two", h=8, g=32, two=2)
                    nc.vector.tensor_tensor(
                        z4l, z4l,
                        vbp[:, 0:NCH_LO, :]
                        .rearrange("p c (h one two) -> p c h one two",
                                   one=1, two=2)
                        .to_broadcast([128, NCH_LO, 8, 32, 2]),
                        op=mybir.AluOpType.mult)
                    z4h = zgh[:, :, 0:512].rearrange(
                        "p c (h g two) -> p c h g two", h=8, g=32, two=2)
                    nc.vector.tensor_tensor(
                        z4h, z4h,
                        vbp[:, NCH_LO:, :]
                        .rearrange("p c (h one two) -> p c h one two",
                                   one=1, two=2)
                        .to_broadcast([128, nch_hi, 8, 32, 2]),
                        op=mybir.AluOpType.mult)
                    zs4 = zsf[:].rearrange("p (h g two) -> p h g two",
                                           h=8, g=32, two=2)
                    nc.vector.tensor_tensor(
                        zs4, zs4,
                        vbsp[:].rearrange("p (h one two) -> p h one two",
                                          one=1, two=2)
                        .to_broadcast([128, 8, 32, 2]),
                        op=mybir.AluOpType.mult)

                    # next bin's gather preps go after the pool multiply
                    if t + 3 < NTILES:
                        zg_lo[t + 3] = emit_prep(t + 3, True)
                        zg_hi[t + 3] = emit_prep(t + 3, False)
                        emit_trigger(t + 3, True)

                    # segment sums on the PE (gathered chunks + self chunk)
                    po = p2pool.tile([128, 512], dt.float32, space="PSUM")
                    ps = p2bpool.tile([128, 8], dt.float32, space="PSUM")
                    for ch in range(nch):
                        sl = sel[:, ch * 128:(ch + 1) * 128]
                        rhs = (zgl[:, ch, 0:512] if ch < NCH_LO
                               else zgh[:, ch - NCH_LO, 0:512])
                        nc.tensor.matmul(po[:], sl, rhs,
                                         start=(ch == 0), stop=False)
                        nc.tensor.matmul(ps[:], sl, vb[:, ch, :],
                                         start=(ch == 0), stop=False)
                    nc.tensor.matmul(po[:], ident_sb[:], zsf[:],
                                     start=False, stop=True)
                    nc.tensor.matmul(ps[:], ident_sb[:], vbs[:],
                                     start=False, stop=True)

                    # out = po / s + bias (eps keeps pad slots finite)
                    ssb = scpool.tile([128, 8], dt.float32, tag="ssb")
                    nc.vector.tensor_scalar_add(ssb[:], ps[:], 1e-30)
                    nc.vector.reciprocal(ssb[:], ssb[:])
                    osb = eopool.tile([128, 512], dt.float32)
                    o4 = osb[:].rearrange("p (h d) -> p h d", d=DH)
                    nc.vector.tensor_tensor(
                        o4, po[:].rearrange("p (h d) -> p h d", d=DH),
                        ssb[:].to_broadcast([128, 8, DH]),
                        op=mybir.AluOpType.mult)
                    oadd = nc.vector.tensor_tensor(
                        osb[:], osb[:], bias_sb[:], op=mybir.AluOpType.add)
                    last_vec_name = oadd.ins.name
                    nc.sync.dma_start(out_d[t * 128:(t + 1) * 128, :], osb[:])
            for p in reversed(p2ctx):
                p.__exit__(None, None, None)
    nc.compile()
    return nc


# ------------------------------------------------------------------- driver
def kernel(x_src, x_dst, edge_src, edge_dst, W, attn_l, attn_r, bias):
    shared, per_core, k_hi, w_panels = _host_prep(
        np.asarray(x_src), np.asarray(x_dst), np.asarray(edge_src),
        np.asarray(edge_dst), np.asarray(W), np.asarray(attn_l),
        np.asarray(attn_r), np.asarray(bias))

    nc = _build_nc(k_hi, w_panels)

    in_maps = []
    for c in range(NCORES):
        in_maps.append({"xT": per_core[c]["xT"], "Wext": shared["Wext"],
                        "bias_rep": shared["bias_rep"],
                        "ident": shared["ident"],
                        "selT": per_core[c]["selT"],
                        "selD": per_core[c]["selD"],
                        "zidx_lo": per_core[c]["zidx_lo"],
                        "zidx_hi": per_core[c]["zidx_hi"]})

    def unperm(out_core, c):
        full = np.zeros((DPC, 512), F32)
        perm = per_core[c]["perm"]
        valid = perm >= 0
        full[perm[valid]] = out_core[np.nonzero(valid)[0]]
        return full

    if os.environ.get("KERNEL_SIM"):
        from concourse.bass_interp import CoreSim
        # require_finite off: the gather's in_ap spans the whole zel table,
        # including never-written pad columns (NaN canary in sim only)
        sim = CoreSim(nc, trace=False, require_finite=False,
                      require_nnan=False)
        cid = int(os.environ.get("KERNEL_SIM_CORE", "0"))
        for name, arr in in_maps[cid].items():
            sim.tensor(name)[:] = arr
        sim.simulate()
        out = unperm(np.array(sim.tensor("out")), cid)
        return np.concatenate([out] * NCORES, 0)  # selected core's slice only

    from concourse.bass_utils import run_bass_kernel_spmd
    res = run_bass_kernel_spmd(nc, in_maps, core_ids=list(range(NCORES)),
                               trace=bool(os.environ.get("KERNEL_TRACE")))
    global LAST_RESULTS
    LAST_RESULTS = res
    return np.concatenate([unperm(r["out"], c)
                           for c, r in enumerate(res.results)], 0)


LAST_RESULTS = None

